# revision 5
# baseline (speedup 1.0000x reference)
"""Trainium2 Bass kernel for nn_CESLayer: y = cos((x+1)*30 @ theta.T + phi).

Math: (x+1)*30 @ theta.T + phi = x @ W + bias, with
  W[k, o] = 30 * theta[o, k],  bias[o] = 30 * sum_k theta[o, k] + phi[o]
and cos(z) = sin(z + pi/2). The ScalarE Sin LUT is only valid on [-pi, pi],
so the kernel computes u = z / (2*pi) via a rescaled matmul (W' = W/2pi,
bias' = (bias + pi/2)/2pi folded in as a rank-1 K=1 matmul), range-reduces
n = rtne(u) with the f32 magic-number trick, and evaluates Sin(2pi*(u - n)).

Data-parallel: 131072 batch rows split across 8 NeuronCores; weight-derived
tensors replicated. x is transposed on-chip (PE transpose via identity) to
serve as the stationary matmul operand.
"""

import os
import sys

for _p in (
    "/root/.axon_site",
    "/root/.axon_site/_ro/trn_rl_repo",
    "/root/.axon_site/_ro/pypackages",
    "/opt/trn_rl_repo",
):
    if os.path.isdir(_p) and _p not in sys.path:
        sys.path.append(_p)

import numpy as np

OMEGA_0 = 30.0
B, IN_DIM, OUT_DIM = 131072, 512, 512
N_CORES = 8
BS = B // N_CORES  # rows per core
P = 128  # partitions
ROW_TILES = BS // P
MAGIC = np.float32(1.5 * 2**23)  # f32 round-to-nearest via add/sub
TWO_PI = 2.0 * np.pi

_cache = {}


def _build(row_tiles=ROW_TILES, num_devices=N_CORES):
    import concourse.mybir as mybir
    import concourse.tile as tile
    from concourse import bacc
    from concourse.masks import make_identity

    bs = row_tiles * P
    f32 = mybir.dt.float32
    Alu = mybir.AluOpType
    nc = bacc.Bacc(
        "TRN2",
        target_bir_lowering=False,
        debug=False,
        enable_asserts=False,
        num_devices=num_devices,
    )
    x_d = nc.dram_tensor("x", [bs, IN_DIM], f32, kind="ExternalInput").ap()
    w_d = nc.dram_tensor("w", [IN_DIM, OUT_DIM], f32, kind="ExternalInput").ap()
    bias_d = nc.dram_tensor("bias_row", [1, OUT_DIM], f32, kind="ExternalInput").ap()
    y_d = nc.dram_tensor("y", [bs, OUT_DIM], f32, kind="ExternalOutput").ap()

    KB = IN_DIM // P  # contraction blocks

    with tile.TileContext(nc) as tc:
        with (
            tc.tile_pool(name="const", bufs=1) as cpool,
            tc.tile_pool(name="xin", bufs=4) as xpool,
            tc.tile_pool(name="xT", bufs=3) as xTpool,
            tc.tile_pool(name="eps", bufs=3) as epool,
            tc.tile_pool(name="yout", bufs=4) as ypool,
            tc.tile_pool(name="psumT", bufs=2, space="PSUM") as pTpool,
            tc.tile_pool(name="psumM", bufs=2, space="PSUM") as pMpool,
        ):
            ident = cpool.tile([P, P], f32)
            make_identity(nc, ident[:])
            ones = cpool.tile([1, P], f32)
            nc.gpsimd.memset(ones[:], 1.0)
            w_sb = cpool.tile([P, KB * OUT_DIM], f32)
            for j in range(KB):
                nc.sync.dma_start(
                    w_sb[:, j * OUT_DIM : (j + 1) * OUT_DIM],
                    w_d[j * P : (j + 1) * P, :],
                )
            bias_sb = cpool.tile([1, OUT_DIM], f32)
            nc.sync.dma_start(bias_sb[:], bias_d[:])

            for i in range(row_tiles):
                xt = xpool.tile([P, IN_DIM], f32)
                nc.sync.dma_start(xt[:], x_d[i * P : (i + 1) * P, :])

                # x tile -> xT via PE transpose (PSUM) + copy back to SBUF
                pT = pTpool.tile([P, IN_DIM], f32)
                for j in range(KB):
                    nc.tensor.transpose(
                        pT[:, j * P : (j + 1) * P], xt[:, j * P : (j + 1) * P], ident[:]
                    )
                xT = xTpool.tile([P, IN_DIM], f32)
                half = IN_DIM // 2
                nc.scalar.copy(xT[:, :half], pT[:, :half])
                nc.vector.tensor_copy(xT[:, half:], pT[:, half:])

                # u = x @ W' + bias' (bias' via rank-1 K=1 matmul, runs first)
                pm = pMpool.tile([P, OUT_DIM], f32)
                nc.tensor.matmul(pm[:], ones[:], bias_sb[:], start=True, stop=False)
                for j in range(KB):
                    nc.tensor.matmul(
                        pm[:],
                        xT[:, j * P : (j + 1) * P],
                        w_sb[:, j * OUT_DIM : (j + 1) * OUT_DIM],
                        start=False,
                        stop=(j == KB - 1),
                    )

                # n = rtne(u); f = u - n in [-0.5, 0.5]; y = sin(2pi * f)
                n_t = epool.tile([P, OUT_DIM], f32)
                nc.vector.tensor_scalar(
                    n_t[:], pm[:], float(MAGIC), float(MAGIC), Alu.add, Alu.subtract
                )
                f_t = epool.tile([P, OUT_DIM], f32)
                nc.vector.tensor_tensor(f_t[:], pm[:], n_t[:], Alu.subtract)
                ys = ypool.tile([P, OUT_DIM], f32)
                nc.scalar.activation(
                    ys[:], f_t[:], mybir.ActivationFunctionType.Sin, scale=float(TWO_PI)
                )
                nc.sync.dma_start(y_d[i * P : (i + 1) * P, :], ys[:])

    nc.compile()
    return nc


def _get_nc():
    if "nc" not in _cache:
        _cache["nc"] = _build()
    return _cache["nc"]


def _host_params(theta, phi):
    w = np.ascontiguousarray(
        (OMEGA_0 / TWO_PI) * theta.T.astype(np.float64)
    ).astype(np.float32)
    bias = (
        (OMEGA_0 * theta.astype(np.float64).sum(axis=1) + phi + np.pi / 2) / TWO_PI
    ).astype(np.float32)
    return w, np.ascontiguousarray(bias.reshape(1, OUT_DIM))


def kernel(x, theta, phi, **run_kwargs):
    from concourse import bass_utils

    nc = _get_nc()
    w, bias_row = _host_params(theta, phi)

    in_maps = [
        {
            "x": np.ascontiguousarray(x[c * BS : (c + 1) * BS]),
            "w": w,
            "bias_row": bias_row,
        }
        for c in range(N_CORES)
    ]
    res = bass_utils.run_bass_kernel_spmd(
        nc, in_maps, core_ids=list(range(N_CORES)), **run_kwargs
    )
    y = np.concatenate([res.results[c]["y"] for c in range(N_CORES)], axis=0)
    if run_kwargs:
        _cache["last_results"] = res
    return y


# revision 10
# speedup vs baseline: 2.2600x; 2.2600x over previous
"""Trainium2 Bass kernel for nn_CESLayer: y = cos((x+1)*30 @ theta.T + phi).

Math: (x+1)*30 @ theta.T + phi = x @ W + bias, with
  W[k, o] = 30 * theta[o, k],  bias[o] = 30 * sum_k theta[o, k] + phi[o]
and cos(z) = sin(z + pi/2). The ScalarE Sin LUT is only valid on [-pi, pi],
so the kernel computes u = z / (2*pi) via a rescaled matmul (W' = W/2pi,
bias' = (bias + pi/2)/2pi folded in as a rank-1 K=1 matmul), range-reduces
n = rtne(u) with the f32 magic-number trick, and evaluates Sin(2pi*(u - n)).

The batch is split across 8 NeuronCores (data parallel), weight tensors
replicated. x is pre-transposed host-side into per-tile [k, b] blocks so
each stationary-operand load is a single contiguous DMA, and matmuls use
float32r (full fp32 storage, reduced-precision PE path at 1 cycle/row for
moving dim >= 256 vs 4 cycles/row for plain fp32).
"""

import os
import sys

for _p in (
    "/root/.axon_site",
    "/root/.axon_site/_ro/trn_rl_repo",
    "/root/.axon_site/_ro/pypackages",
    "/opt/trn_rl_repo",
):
    if os.path.isdir(_p) and _p not in sys.path:
        sys.path.append(_p)

import numpy as np

OMEGA_0 = 30.0
B, IN_DIM, OUT_DIM = 131072, 512, 512
N_CORES = 8
BS = B // N_CORES  # rows per core
P = 128  # partitions
ROW_TILES = BS // P
KB = IN_DIM // P  # contraction blocks
MAGIC = np.float32(1.5 * 2**23)  # f32 round-to-nearest via add/sub
TWO_PI = 2.0 * np.pi

_cache = {}


def _build(row_tiles=ROW_TILES, num_devices=N_CORES):
    import concourse.mybir as mybir
    import concourse.tile as tile
    from concourse import bacc

    f32 = mybir.dt.float32
    f32r = mybir.dt.float32r
    Alu = mybir.AluOpType
    nc = bacc.Bacc(
        "TRN2",
        target_bir_lowering=False,
        debug=False,
        enable_asserts=False,
        num_devices=num_devices,
    )
    # xt[i, k, f] = x[i*128 + f%128, (f//128)*128 + k] (pre-transposed tiles)
    xt_d = nc.dram_tensor(
        "xt", [row_tiles, P, IN_DIM], f32r, kind="ExternalInput"
    ).ap()
    w_d = nc.dram_tensor("w", [IN_DIM, OUT_DIM], f32r, kind="ExternalInput").ap()
    bias_d = nc.dram_tensor("bias_row", [1, OUT_DIM], f32r, kind="ExternalInput").ap()
    ones_d = nc.dram_tensor("ones_row", [1, P], f32r, kind="ExternalInput").ap()
    y_d = nc.dram_tensor("y", [row_tiles * P, OUT_DIM], f32, kind="ExternalOutput").ap()

    with tile.TileContext(nc) as tc:
        with (
            tc.tile_pool(name="const", bufs=1) as cpool,
            tc.tile_pool(name="xin", bufs=6) as xpool,
            tc.tile_pool(name="eps", bufs=4) as epool,
            tc.tile_pool(name="yout", bufs=6) as ypool,
            tc.tile_pool(name="psumM", bufs=4, space="PSUM") as pMpool,
        ):
            ones = cpool.tile([1, P], f32r)
            nc.sync.dma_start(ones[:], ones_d[:])
            w_sb = cpool.tile([P, KB * OUT_DIM], f32r)
            for j in range(KB):
                nc.sync.dma_start(
                    w_sb[:, j * OUT_DIM : (j + 1) * OUT_DIM],
                    w_d[j * P : (j + 1) * P, :],
                )
            bias_sb = cpool.tile([1, OUT_DIM], f32r)
            nc.sync.dma_start(bias_sb[:], bias_d[:])

            for i in range(row_tiles):
                xT = xpool.tile([P, IN_DIM], f32r)
                nc.sync.dma_start(xT[:], xt_d[i])

                # u = x @ W' + bias' (bias' via rank-1 K=1 matmul, runs first)
                pm = pMpool.tile([P, OUT_DIM], f32)
                nc.tensor.matmul(pm[:], ones[:], bias_sb[:], start=True, stop=False)
                for j in range(KB):
                    nc.tensor.matmul(
                        pm[:],
                        xT[:, j * P : (j + 1) * P],
                        w_sb[:, j * OUT_DIM : (j + 1) * OUT_DIM],
                        start=False,
                        stop=(j == KB - 1),
                    )

                # n = rtne(u); f = u - n in [-0.5, 0.5]; y = sin(2pi * f)
                n_t = epool.tile([P, OUT_DIM], f32)
                nc.vector.tensor_scalar(
                    n_t[:], pm[:], float(MAGIC), float(MAGIC), Alu.add, Alu.subtract
                )
                f_t = epool.tile([P, OUT_DIM], f32)
                nc.vector.tensor_tensor(f_t[:], pm[:], n_t[:], Alu.subtract)
                ys = ypool.tile([P, OUT_DIM], f32)
                nc.scalar.activation(
                    ys[:], f_t[:], mybir.ActivationFunctionType.Sin, scale=float(TWO_PI)
                )
                nc.sync.dma_start(y_d[i * P : (i + 1) * P, :], ys[:])

    nc.compile()
    return nc


def _get_nc():
    if "nc" not in _cache:
        _cache["nc"] = _build()
    return _cache["nc"]


def _host_params(theta, phi):
    w = np.ascontiguousarray(
        (OMEGA_0 / TWO_PI) * theta.T.astype(np.float64)
    ).astype(np.float32)
    bias = (
        (OMEGA_0 * theta.astype(np.float64).sum(axis=1) + phi + np.pi / 2) / TWO_PI
    ).astype(np.float32)
    return w, np.ascontiguousarray(bias.reshape(1, OUT_DIM))


def _pretranspose(x_shard, row_tiles=ROW_TILES):
    # [T*128, KB*128] -> [T, P(k), KB*128(f=j*128+b)] with
    # xt[i, k, j*128 + b] = x[i*128 + b, j*128 + k]
    x5 = x_shard.reshape(row_tiles, P, KB, P)  # [i, b, j, k]
    return np.ascontiguousarray(x5.transpose(0, 3, 2, 1)).reshape(
        row_tiles, P, IN_DIM
    )


def kernel(x, theta, phi, **run_kwargs):
    from concourse import bass_utils

    nc = _get_nc()
    w, bias_row = _host_params(theta, phi)

    in_maps = [
        {
            "xt": _pretranspose(x[c * BS : (c + 1) * BS]),
            "w": w,
            "bias_row": bias_row,
            "ones_row": np.ones((1, P), np.float32),
        }
        for c in range(N_CORES)
    ]
    res = bass_utils.run_bass_kernel_spmd(
        nc, in_maps, core_ids=list(range(N_CORES)), **run_kwargs
    )
    y = np.concatenate([res.results[c]["y"] for c in range(N_CORES)], axis=0)
    if run_kwargs:
        _cache["last_results"] = res
    return y


# revision 16
# speedup vs baseline: 3.2935x; 1.4573x over previous
"""Trainium2 Bass kernel for nn_CESLayer: y = cos((x+1)*30 @ theta.T + phi).

Math: (x+1)*30 @ theta.T + phi = x @ W + bias, with
  W[k, o] = 30 * theta[o, k],  bias[o] = 30 * sum_k theta[o, k] + phi[o]
and cos(z) = sin(z + pi/2). The ScalarE Sin LUT is only valid on [-pi, pi],
so the kernel computes u = z / (2*pi) via a rescaled matmul (W' = W/2pi),
then f0 = (u + bias' + 0.5) mod 1.0 in one VectorE op (bias' as a
per-partition scalar), and y = Sin(2*pi*f0 - pi) on ScalarE.

Layout: the output is computed TRANSPOSED — psum tiles are [o, b] so the
per-output bias rides the per-partition scalar operands; the host
un-transposes at the end. x is pre-transposed host-side into [k, b] group
tiles so every DMA is contiguous and no on-chip transpose is needed.
Batch is split across 8 NeuronCores (data parallel), weights replicated.

Matmul runs in fp16 (1 PE cycle/row vs 4 for fp32, 2 for float32r) and the
output is stored fp16 (upcast on host), halving write traffic. The phase is
accumulated in fp32 PSUM; fp16 quantization of x/W contributes ~1e-3
relative error, well inside the gate.
"""

import os
import sys

for _p in (
    "/root/.axon_site",
    "/root/.axon_site/_ro/trn_rl_repo",
    "/root/.axon_site/_ro/pypackages",
    "/opt/trn_rl_repo",
):
    if os.path.isdir(_p) and _p not in sys.path:
        sys.path.append(_p)

import numpy as np

OMEGA_0 = 30.0
B, IN_DIM, OUT_DIM = 131072, 512, 512
N_CORES = 8
BS = B // N_CORES  # rows per core
P = 128  # partitions
KB = IN_DIM // P  # contraction blocks
OB = OUT_DIM // P  # output blocks
GW = 512  # batch columns per group (moving free dim)
GROUPS = BS // GW
MAGIC = float(np.float32(1.5 * 2**23))  # f32 round-to-nearest via add/sub
TWO_PI = 2.0 * np.pi

# mm_dt: matmul operand dtype ("f16" or "f32r"); out_dt: DMA'd output dtype;
# epilogue: "mod" (1 DVE op) or "magic" (rank-1 bias matmul + 2 DVE ops)
CONFIG = {
    "mm_dt": os.environ.get("K_MM_DT", "f16"),
    "out_dt": os.environ.get("K_OUT_DT", "f16"),
    "epilogue": os.environ.get("K_EPILOGUE", "magic"),
}

_cache = {}


def _np_dt(name):
    return {"f16": np.float16, "f32": np.float32, "f32r": np.float32}[name]


def _build(groups=GROUPS, num_devices=N_CORES, cfg=None):
    import concourse.mybir as mybir
    import concourse.tile as tile
    from concourse import bacc

    cfg = dict(CONFIG if cfg is None else cfg)
    f32 = mybir.dt.float32
    mm_dt = {"f16": mybir.dt.float16, "f32r": mybir.dt.float32r}[cfg["mm_dt"]]
    out_dt = {"f16": mybir.dt.float16, "f32": mybir.dt.float32}[cfg["out_dt"]]
    Alu = mybir.AluOpType
    Act = mybir.ActivationFunctionType
    bs = groups * GW

    nc = bacc.Bacc(
        "TRN2",
        target_bir_lowering=False,
        debug=False,
        enable_asserts=False,
        num_devices=num_devices,
    )
    # xt[g, k, b] = x[g*GW + b, k] (per-group transposed x)
    xt_d = nc.dram_tensor("xt", [groups, IN_DIM, GW], mm_dt, kind="ExternalInput").ap()
    # w[k, o] = 30 * theta[o, k] / (2*pi)
    w_d = nc.dram_tensor("w", [IN_DIM, OUT_DIM], mm_dt, kind="ExternalInput").ap()
    # bias1[p, ob] = bias'[ob*128 + p] + 0.5 (f32, per-partition scalar)
    bias1_d = nc.dram_tensor("bias1", [P, OB], f32, kind="ExternalInput").ap()
    # magic-path inputs (tiny; always declared, only used by that epilogue)
    bias_row_d = nc.dram_tensor("bias_row", [1, OUT_DIM], mm_dt, kind="ExternalInput").ap()
    ones_d = nc.dram_tensor("ones_row", [1, GW], mm_dt, kind="ExternalInput").ap()
    bias2_d = nc.dram_tensor("bias2", [P, OB], f32, kind="ExternalInput").ap()
    # yt[o, b'] = y[b', o] transposed output
    yt_d = nc.dram_tensor("yt", [OUT_DIM, bs], out_dt, kind="ExternalOutput").ap()

    with tile.TileContext(nc) as tc:
        with (
            tc.tile_pool(name="const", bufs=1) as cpool,
            tc.tile_pool(name="xin", bufs=3) as xpool,
            tc.tile_pool(name="eps", bufs=8) as epool,
            tc.tile_pool(name="yout", bufs=8) as ypool,
            tc.tile_pool(name="psumM", bufs=6, space="PSUM") as pMpool,
        ):
            w_sb = cpool.tile([P, KB * OUT_DIM], mm_dt)
            for j in range(KB):
                nc.sync.dma_start(
                    w_sb[:, j * OUT_DIM : (j + 1) * OUT_DIM],
                    w_d[j * P : (j + 1) * P, :],
                )
            bias1_sb = cpool.tile([P, OB], f32)
            nc.sync.dma_start(bias1_sb[:], bias1_d[:])
            bias2_sb = cpool.tile([P, OB], f32)
            nc.sync.dma_start(bias2_sb[:], bias2_d[:])
            biasrow_sb = cpool.tile([1, OUT_DIM], mm_dt)
            nc.sync.dma_start(biasrow_sb[:], bias_row_d[:])
            ones_sb = cpool.tile([1, GW], mm_dt)
            nc.sync.dma_start(ones_sb[:], ones_d[:])

            for g in range(groups):
                xg = xpool.tile([P, KB * GW], mm_dt)
                for j in range(KB):
                    nc.sync.dma_start(
                        xg[:, j * GW : (j + 1) * GW],
                        xt_d[g, j * P : (j + 1) * P, :],
                    )
                for ob in range(OB):
                    pm = pMpool.tile([P, GW], f32)
                    if cfg["epilogue"] == "magic":
                        nc.tensor.matmul(
                            pm[:],
                            biasrow_sb[:, ob * P : (ob + 1) * P],
                            ones_sb[:],
                            start=True,
                            stop=False,
                        )
                    for j in range(KB):
                        nc.tensor.matmul(
                            pm[:],
                            w_sb[:, j * OUT_DIM + ob * P : j * OUT_DIM + (ob + 1) * P],
                            xg[:, j * GW : (j + 1) * GW],
                            start=(j == 0 and cfg["epilogue"] != "magic"),
                            stop=(j == KB - 1),
                        )
                    ys = ypool.tile([P, GW], out_dt)
                    if cfg["epilogue"] == "mod":
                        # f0 = (u + b' + 0.5) mod 1.0 in [0, 1)
                        f0 = epool.tile([P, GW], f32)
                        nc.vector.tensor_scalar(
                            f0[:],
                            pm[:],
                            bias1_sb[:, ob : ob + 1],
                            1.0,
                            Alu.add,
                            Alu.mod,
                        )
                        # y = sin(2*pi*f0 - pi); bias2 holds -pi per partition
                        nc.scalar.activation(
                            ys[:],
                            f0[:],
                            Act.Sin,
                            scale=float(TWO_PI),
                            bias=bias2_sb[:, ob : ob + 1],
                        )
                    else:
                        # n = rtne(u); f = u - n; y = sin(2*pi*f + 2*pi*b')
                        n_t = epool.tile([P, GW], f32)
                        nc.vector.tensor_scalar(
                            n_t[:], pm[:], MAGIC, MAGIC, Alu.add, Alu.subtract
                        )
                        f_t = epool.tile([P, GW], f32)
                        nc.vector.tensor_tensor(f_t[:], pm[:], n_t[:], Alu.subtract)
                        nc.scalar.activation(
                            ys[:],
                            f_t[:],
                            Act.Sin,
                            scale=float(TWO_PI),
                            bias=bias2_sb[:, ob : ob + 1],
                        )
                    nc.sync.dma_start(
                        yt_d[ob * P : (ob + 1) * P, g * GW : (g + 1) * GW], ys[:]
                    )

    nc.compile()
    return nc


def _get_nc():
    if "nc" not in _cache:
        _cache["nc"] = _build()
    return _cache["nc"]


def _host_params(theta, phi, cfg=None):
    cfg = dict(CONFIG if cfg is None else cfg)
    mm_np = _np_dt(cfg["mm_dt"])
    w = np.ascontiguousarray(
        (OMEGA_0 / TWO_PI) * theta.T.astype(np.float64)
    ).astype(mm_np)
    bias = (
        (OMEGA_0 * theta.astype(np.float64).sum(axis=1) + phi + np.pi / 2) / TWO_PI
    ).astype(np.float32)
    # +16 keeps the mod input positive (floor vs trunc mod then agree);
    # |u + b' + 0.5| < 13 for this problem's scales
    bias1 = np.ascontiguousarray((bias + np.float32(16.5)).reshape(OB, P).T)
    # magic path: bias folded into matmul needs bias mod 1 to keep fp16 exact-ish
    bias_red = (bias - np.round(bias.astype(np.float64))).astype(np.float32)
    bias_row = np.ascontiguousarray(bias_red.reshape(1, OUT_DIM)).astype(mm_np)
    if cfg["epilogue"] == "mod":
        # ACT bias for sin(2*pi*f0 - pi)
        bias2 = np.full((P, OB), -np.pi, np.float32)
    else:
        bias2 = np.ascontiguousarray(
            (TWO_PI * (bias_red - bias_row.astype(np.float32))).reshape(OB, P).T
        ).astype(np.float32)
    return w, bias1, bias_row, bias2


def _pretranspose(x_shard, groups=GROUPS, cfg=None):
    cfg = dict(CONFIG if cfg is None else cfg)
    mm_np = _np_dt(cfg["mm_dt"])
    xg = x_shard.astype(mm_np).reshape(groups, GW, IN_DIM)
    return np.ascontiguousarray(xg.transpose(0, 2, 1))


def kernel(x, theta, phi, **run_kwargs):
    from concourse import bass_utils

    nc = _get_nc()
    w, bias1, bias_row, bias2 = _host_params(theta, phi)
    ones_row = np.ones((1, GW), _np_dt(CONFIG["mm_dt"]))

    in_maps = [
        {
            "xt": _pretranspose(x[c * BS : (c + 1) * BS]),
            "w": w,
            "bias1": bias1,
            "bias_row": bias_row,
            "bias2": bias2,
            "ones_row": ones_row,
        }
        for c in range(N_CORES)
    ]
    res = bass_utils.run_bass_kernel_spmd(
        nc, in_maps, core_ids=list(range(N_CORES)), **run_kwargs
    )
    y = np.concatenate(
        [res.results[c]["yt"].T.astype(np.float32) for c in range(N_CORES)], axis=0
    )
    if run_kwargs:
        _cache["last_results"] = res
    return y


# revision 17
# speedup vs baseline: 3.3554x; 1.0188x over previous
"""Trainium2 Bass kernel for nn_CESLayer: y = cos((x+1)*30 @ theta.T + phi).

Math: (x+1)*30 @ theta.T + phi = x @ W + bias, with
  W[k, o] = 30 * theta[o, k],  bias[o] = 30 * sum_k theta[o, k] + phi[o]
and cos(z) = sin(z + pi/2). The ScalarE Sin LUT is only valid on [-pi, pi],
so the kernel computes u = z / (2*pi) via a rescaled matmul (W' = W/2pi),
then f0 = (u + bias' + 0.5) mod 1.0 in one VectorE op (bias' as a
per-partition scalar), and y = Sin(2*pi*f0 - pi) on ScalarE.

Layout: the output is computed TRANSPOSED — psum tiles are [o, b] so the
per-output bias rides the per-partition scalar operands; the host
un-transposes at the end. x is pre-transposed host-side into [k, b] group
tiles so every DMA is contiguous and no on-chip transpose is needed.
Batch is split across 8 NeuronCores (data parallel), weights replicated.

Matmul runs in fp16 (1 PE cycle/row vs 4 for fp32, 2 for float32r) and the
output is stored fp16 (upcast on host), halving write traffic. The phase is
accumulated in fp32 PSUM; fp16 quantization of x/W contributes ~1e-3
relative error, well inside the gate.
"""

import os
import sys

for _p in (
    "/root/.axon_site",
    "/root/.axon_site/_ro/trn_rl_repo",
    "/root/.axon_site/_ro/pypackages",
    "/opt/trn_rl_repo",
):
    if os.path.isdir(_p) and _p not in sys.path:
        sys.path.append(_p)

import numpy as np

OMEGA_0 = 30.0
B, IN_DIM, OUT_DIM = 131072, 512, 512
N_CORES = 8
BS = B // N_CORES  # rows per core
P = 128  # partitions
KB = IN_DIM // P  # contraction blocks
OB = OUT_DIM // P  # output blocks
GW = 512  # batch columns per group (moving free dim)
GROUPS = BS // GW
MAGIC = float(np.float32(1.5 * 2**23))  # f32 round-to-nearest via add/sub
TWO_PI = 2.0 * np.pi

# mm_dt: matmul operand dtype ("f16" or "f32r"); out_dt: DMA'd output dtype;
# epilogue: "mod" (1 DVE op) or "magic" (rank-1 bias matmul + 2 DVE ops)
CONFIG = {
    "mm_dt": os.environ.get("K_MM_DT", "f16"),
    "out_dt": os.environ.get("K_OUT_DT", "f16"),
    "epilogue": os.environ.get("K_EPILOGUE", "magic"),
}

_cache = {}


def _np_dt(name):
    return {"f16": np.float16, "bf16": __import__("ml_dtypes").bfloat16, "f32": np.float32, "f32r": np.float32}[name]


def _build(groups=GROUPS, num_devices=N_CORES, cfg=None):
    import concourse.mybir as mybir
    import concourse.tile as tile
    from concourse import bacc

    cfg = dict(CONFIG if cfg is None else cfg)
    f32 = mybir.dt.float32
    mm_dt = {"f16": mybir.dt.float16, "bf16": mybir.dt.bfloat16, "f32r": mybir.dt.float32r}[cfg["mm_dt"]]
    out_dt = {"f16": mybir.dt.float16, "f32": mybir.dt.float32}[cfg["out_dt"]]
    Alu = mybir.AluOpType
    Act = mybir.ActivationFunctionType
    bs = groups * GW

    nc = bacc.Bacc(
        "TRN2",
        target_bir_lowering=False,
        debug=False,
        enable_asserts=False,
        num_devices=num_devices,
    )
    # xt[g, k, b] = x[g*GW + b, k] (per-group transposed x)
    xt_d = nc.dram_tensor("xt", [groups, IN_DIM, GW], mm_dt, kind="ExternalInput").ap()
    # w[k, o] = 30 * theta[o, k] / (2*pi)
    w_d = nc.dram_tensor("w", [IN_DIM, OUT_DIM], mm_dt, kind="ExternalInput").ap()
    # bias1[p, ob] = bias'[ob*128 + p] + 0.5 (f32, per-partition scalar)
    bias1_d = nc.dram_tensor("bias1", [P, OB], f32, kind="ExternalInput").ap()
    # magic-path inputs (tiny; always declared, only used by that epilogue)
    bias_row_d = nc.dram_tensor("bias_row", [1, OUT_DIM], mm_dt, kind="ExternalInput").ap()
    ones_d = nc.dram_tensor("ones_row", [1, GW], mm_dt, kind="ExternalInput").ap()
    bias2_d = nc.dram_tensor("bias2", [P, OB], f32, kind="ExternalInput").ap()
    # yt[o, b'] = y[b', o] transposed output
    yt_d = nc.dram_tensor("yt", [OUT_DIM, bs], out_dt, kind="ExternalOutput").ap()

    with tile.TileContext(nc) as tc:
        with (
            tc.tile_pool(name="const", bufs=1) as cpool,
            tc.tile_pool(name="xin", bufs=3) as xpool,
            tc.tile_pool(name="eps", bufs=8) as epool,
            tc.tile_pool(name="yout", bufs=8) as ypool,
            tc.tile_pool(name="psumM", bufs=6, space="PSUM") as pMpool,
        ):
            w_sb = cpool.tile([P, KB * OUT_DIM], mm_dt)
            for j in range(KB):
                nc.sync.dma_start(
                    w_sb[:, j * OUT_DIM : (j + 1) * OUT_DIM],
                    w_d[j * P : (j + 1) * P, :],
                )
            bias1_sb = cpool.tile([P, OB], f32)
            nc.sync.dma_start(bias1_sb[:], bias1_d[:])
            bias2_sb = cpool.tile([P, OB], f32)
            nc.sync.dma_start(bias2_sb[:], bias2_d[:])
            biasrow_sb = cpool.tile([1, OUT_DIM], mm_dt)
            nc.sync.dma_start(biasrow_sb[:], bias_row_d[:])
            ones_sb = cpool.tile([1, GW], mm_dt)
            nc.sync.dma_start(ones_sb[:], ones_d[:])

            for g in range(groups):
                xg = xpool.tile([P, KB * GW], mm_dt)
                for j in range(KB):
                    nc.sync.dma_start(
                        xg[:, j * GW : (j + 1) * GW],
                        xt_d[g, j * P : (j + 1) * P, :],
                    )
                for ob in range(OB):
                    pm = pMpool.tile([P, GW], f32)
                    if cfg["epilogue"] == "magic":
                        nc.tensor.matmul(
                            pm[:],
                            biasrow_sb[:, ob * P : (ob + 1) * P],
                            ones_sb[:],
                            start=True,
                            stop=False,
                        )
                    for j in range(KB):
                        nc.tensor.matmul(
                            pm[:],
                            w_sb[:, j * OUT_DIM + ob * P : j * OUT_DIM + (ob + 1) * P],
                            xg[:, j * GW : (j + 1) * GW],
                            start=(j == 0 and cfg["epilogue"] != "magic"),
                            stop=(j == KB - 1),
                        )
                    ys = ypool.tile([P, GW], out_dt)
                    if cfg["epilogue"] == "mod":
                        # f0 = (u + b' + 0.5) mod 1.0 in [0, 1)
                        f0 = epool.tile([P, GW], f32)
                        nc.vector.tensor_scalar(
                            f0[:],
                            pm[:],
                            bias1_sb[:, ob : ob + 1],
                            1.0,
                            Alu.add,
                            Alu.mod,
                        )
                        # y = sin(2*pi*f0 - pi); bias2 holds -pi per partition
                        nc.scalar.activation(
                            ys[:],
                            f0[:],
                            Act.Sin,
                            scale=float(TWO_PI),
                            bias=bias2_sb[:, ob : ob + 1],
                        )
                    else:
                        # n = rtne(u); f = u - n; y = sin(2*pi*f + 2*pi*b')
                        n_t = epool.tile([P, GW], f32)
                        nc.vector.tensor_scalar(
                            n_t[:], pm[:], MAGIC, MAGIC, Alu.add, Alu.subtract
                        )
                        f_t = epool.tile([P, GW], f32)
                        nc.vector.tensor_tensor(f_t[:], pm[:], n_t[:], Alu.subtract)
                        nc.scalar.activation(
                            ys[:],
                            f_t[:],
                            Act.Sin,
                            scale=float(TWO_PI),
                            bias=bias2_sb[:, ob : ob + 1],
                        )
                    nc.sync.dma_start(
                        yt_d[ob * P : (ob + 1) * P, g * GW : (g + 1) * GW], ys[:]
                    )

    nc.compile()
    return nc


def _get_nc():
    if "nc" not in _cache:
        _cache["nc"] = _build()
    return _cache["nc"]


def _host_params(theta, phi, cfg=None):
    cfg = dict(CONFIG if cfg is None else cfg)
    mm_np = _np_dt(cfg["mm_dt"])
    w = np.ascontiguousarray(
        (OMEGA_0 / TWO_PI) * theta.T.astype(np.float64)
    ).astype(mm_np)
    bias = (
        (OMEGA_0 * theta.astype(np.float64).sum(axis=1) + phi + np.pi / 2) / TWO_PI
    ).astype(np.float32)
    # +16 keeps the mod input positive (floor vs trunc mod then agree);
    # |u + b' + 0.5| < 13 for this problem's scales
    bias1 = np.ascontiguousarray((bias + np.float32(16.5)).reshape(OB, P).T)
    # magic path: bias folded into matmul needs bias mod 1 to keep fp16 exact-ish
    bias_red = (bias - np.round(bias.astype(np.float64))).astype(np.float32)
    bias_row = np.ascontiguousarray(bias_red.reshape(1, OUT_DIM)).astype(mm_np)
    if cfg["epilogue"] == "mod":
        # ACT bias for sin(2*pi*f0 - pi)
        bias2 = np.full((P, OB), -np.pi, np.float32)
    else:
        bias2 = np.ascontiguousarray(
            (TWO_PI * (bias_red - bias_row.astype(np.float32))).reshape(OB, P).T
        ).astype(np.float32)
    return w, bias1, bias_row, bias2


def _pretranspose(x_shard, groups=GROUPS, cfg=None):
    cfg = dict(CONFIG if cfg is None else cfg)
    mm_np = _np_dt(cfg["mm_dt"])
    xg = x_shard.astype(mm_np).reshape(groups, GW, IN_DIM)
    return np.ascontiguousarray(xg.transpose(0, 2, 1))


def kernel(x, theta, phi, **run_kwargs):
    from concourse import bass_utils

    nc = _get_nc()
    w, bias1, bias_row, bias2 = _host_params(theta, phi)
    ones_row = np.ones((1, GW), _np_dt(CONFIG["mm_dt"]))

    in_maps = [
        {
            "xt": _pretranspose(x[c * BS : (c + 1) * BS]),
            "w": w,
            "bias1": bias1,
            "bias_row": bias_row,
            "bias2": bias2,
            "ones_row": ones_row,
        }
        for c in range(N_CORES)
    ]
    res = bass_utils.run_bass_kernel_spmd(
        nc, in_maps, core_ids=list(range(N_CORES)), **run_kwargs
    )
    y = np.concatenate(
        [res.results[c]["yt"].T.astype(np.float32) for c in range(N_CORES)], axis=0
    )
    if run_kwargs:
        _cache["last_results"] = res
    return y
